# revision 33
# baseline (speedup 1.0000x reference)
"""Fused multi-head attention block (B=2, N=4096, C=768, H=12, D=64) for 8
Trainium2 NeuronCores.

Sharding: core c -> (batch b = c // 4, head-group g = c % 4, heads
[3g, 3g+1, 3g+2]).  Megatron-style: qkv weights column-split per head
group, proj weights row-split; each core emits a partial [N, C] output
and the host sums the 4 partials per batch and adds proj_b.

bf16 operand pipeline (PSUM accumulation stays fp32):
  phase 1: k/v^T projection from host-pre-transposed x^T (bf16), x tiles
           kept resident in SBUF for reuse by the phase-2 q projection.
           wqkv column groups (all 128 wide, host-packed):
             m0 [q_a*s | q_b*s]   m1 [k_a | k_b]   m2 [q_c*s | q_c*s]
             m3 [k_c  | v_c ]     m4 [v_a | v_b]
           v^T transposed on-PE (matmul vs identity) into per-key-block
           Vaug slabs [128, 65] whose column 64 is 1.0.
  phase 2: per query-chunk (512 cols), heads processed sequentially so
           each head's softmax-normalize (DVE reciprocal) hides under the
           next head's S/AV matmuls:
             S^T block = kt_blk^T @ qt_chunk (bf16, PSUM fp32)
             P = exp(S) on ACT straight out of PSUM -> bf16
             O'^T[65,512] += Vaug_blk^T @ P^T_blk (PSUM accumulate);
             row 64 = softmax denominator.
           Normalize: DVE reciprocal -> PE outer-product broadcast ->
           DVE multiply -> otp bf16.  proj: y[128,:] = sum_h O^T_h @ pw_h,
           PSUM -> SBUF -> DRAM.
"""

import sys

sys.path.insert(0, "/opt/trn_rl_repo")

from contextlib import ExitStack

import numpy as np
import ml_dtypes

import concourse.bacc as bacc
import concourse.bass as bass
import concourse.mybir as mybir
import concourse.tile as tile

B, N, C, H, D = 2, 4096, 768, 12, 64
SCALE = D ** -0.5
F32 = mybir.dt.float32
F32R = mybir.dt.float32r
BF16 = mybir.dt.bfloat16

MM_DT = BF16
AT_DT = BF16
NP_BF16 = ml_dtypes.bfloat16


def build_nc(seq=N):
    """Build + compile the per-core SPMD program."""
    NS = seq // 512   # 512-wide seq chunks
    NB = seq // 128   # 128-wide kj blocks

    nc = bacc.Bacc("TRN2", target_bir_lowering=False, debug=False, num_devices=8)
    xt = nc.dram_tensor("xt", [768, seq], MM_DT, kind="ExternalInput").ap()
    wqkv = nc.dram_tensor("wqkv", [768, 640], MM_DT, kind="ExternalInput").ap()
    wb = nc.dram_tensor("wb", [128, 5], F32, kind="ExternalInput").ap()
    pwt = nc.dram_tensor("pwt", [384, 768], MM_DT, kind="ExternalInput").ap()
    ident = nc.dram_tensor("ident", [128, 64], MM_DT, kind="ExternalInput").ap()
    y = nc.dram_tensor("y", [seq, 768], F32, kind="ExternalOutput").ap()

    # wqkv column groups (all 128 wide; 640 cols with last 64 unused pad)
    MOFF = [0, 128, 256, 384, 512]

    with tile.TileContext(nc) as tc, ExitStack() as ctx:
        const = ctx.enter_context(tc.tile_pool(name="const", bufs=1))
        big = ctx.enter_context(tc.tile_pool(name="big", bufs=1))
        xp_pool = ctx.enter_context(tc.tile_pool(name="xp", bufs=1))
        pt_pool = ctx.enter_context(tc.tile_pool(name="pt", bufs=12))
        # dedicated pools per normalize temp: a shared ring would chain
        # rinv(h+1)'s allocation behind invb(h)'s last reader (the otp
        # multiply), serializing the reciprocal into the previous head's
        # late normalize chain
        rv_pool = ctx.enter_context(tc.tile_pool(name="rv", bufs=2))
        rr_pool = ctx.enter_context(tc.tile_pool(name="rr", bufs=2))
        ib_pool = ctx.enter_context(tc.tile_pool(name="ib", bufs=2))

        y_pool = ctx.enter_context(tc.tile_pool(name="yp", bufs=2))
        vst_pool = ctx.enter_context(tc.tile_pool(name="vst", bufs=2))
        stp = ctx.enter_context(tc.tile_pool(name="stp", bufs=2, space="PSUM"))
        # 4 oacc slots: chunk Q+1's h0 accumulator must not wait on chunk
        # Q's h0 normalize (deadlock via the in-order PE queue otherwise)
        pso = ctx.enter_context(tc.tile_pool(name="pso", bufs=4, space="PSUM"))

        def copyback(dst, srcap, bias_ap):
            # psum -> sbuf copy with per-partition bias on DVE
            nc.vector.tensor_scalar_add(dst, srcap, bias_ap)

        # ---- PE warmup: get HAM to full clock during the DMA shadow ----
        wscr = const.tile([128, 512], MM_DT, tag="wscr")
        nc.vector.memset(wscr[:], 0.0)
        for i in range(36):
            pw_ = stp.tile([128, 512], F32, tag="stp", name="wup")
            nc.tensor.matmul(pw_[:], lhsT=wscr[:, 0:128], rhs=wscr[:],
                             start=True, stop=True)

        # ---- constants ----
        w_sb = []
        for cch in range(6):
            row = []
            for m in range(5):
                t = const.tile([128, 128], MM_DT, tag=f"w{cch}_{m}",
                               name=f"w{cch}_{m}")
                nc.sync.dma_start(
                    t[:], wqkv[cch * 128:(cch + 1) * 128,
                               m * 128:(m + 1) * 128])
                row.append(t)
            w_sb.append(row)
        wb_sb = const.tile([128, 5], F32, tag="wb")
        nc.sync.dma_start(wb_sb[:], wb[:])
        id_sb = const.tile([128, 64], MM_DT, tag="id")
        nc.gpsimd.dma_start(id_sb[:], ident[:])
        pw_sb = []
        for h in range(3):
            t = const.tile([128, 768], MM_DT, tag=f"pw{h}", name=f"pw{h}")
            nc.sync.dma_start(t[:], pwt[h * 128:(h + 1) * 128, :])
            pw_sb.append(t)
        ones_sb = const.tile([128, 64], F32R, tag="ones")
        nc.vector.memset(ones_sb[:].bitcast(F32), 1.0)

        # ---- persistent qkv^T tensors ----
        q01 = big.tile([128, seq], AT_DT, tag="q01")
        q2 = big.tile([128, seq], AT_DT, tag="q2")
        ka = big.tile([128, seq], AT_DT, tag="ka")
        kb = big.tile([128, seq], AT_DT, tag="kb")
        kc = big.tile([128, seq], AT_DT, tag="kc")
        nc.vector.memset(ka[64:128, :], 0.0)
        nc.vector.memset(kb[0:64, :], 0.0)
        nc.vector.memset(kc[64:128, :], 0.0)
        # Vaug slabs padded to 128 cols (col 64 = ones row-sum column, cols
        # 65:127 harmless pad) so AV weight loads qualify for FWL (128 cols)
        vaug = [big.tile([128, NB * 128], AT_DT, tag=f"va{h}", name=f"va{h}")
                for h in range(3)]
        otp = [big.tile([128, 512], MM_DT, tag=f"otp{h}", name=f"otp{h}")
               for h in range(3)]
        for h in range(3):
            nc.vector.memset(otp[h][64:128, :], 0.0)
        for h in range(3):
            nc.vector.memset(vaug[h][:], 1.0)

        # ---- helpers shared by phase 1 (fused chunk 0) and phase 2 ----
        heads = [(ka, q01), (kb, q01), (kc, q2)]

        def norm_start(oacc):
            # DVE-only part: reciprocal of the softmax sums (slow, ~3.3us)
            # -- runs on DVE without blocking the in-order PE queue
            rinv = rv_pool.tile([128, 512], F32, tag="rinv")
            nc.vector.reciprocal(rinv[64:65, :], oacc[64:65, :])
            rinvr = rr_pool.tile([128, 512], F32R, tag="rinvr", name="rinvr")
            nc.vector.tensor_copy(rinvr[64:65, :], rinv[64:65, :])
            return rinvr

        def norm_finish(h, oacc, rinvr):
            # PE broadcast of 1/sums + DVE multiply -> otp[h] (bf16).
            # Emitted ~4 blocks into the NEXT head so the in-order PE queue
            # reaches it only after the reciprocal chain has finished.
            psb = stp.tile([128, 512], F32, tag="stp", name="psb")
            nc.tensor.matmul(
                psb[0:64, :],
                lhsT=ones_sb[64:65, :],
                rhs=rinvr[64:65, :],
                start=True,
                stop=True,
            )
            invb = ib_pool.tile([64, 512], F32, tag="invb", name="invb")
            nc.vector.tensor_copy(invb[:], psb[0:64, :])
            nc.vector.tensor_mul(otp[h][0:64, :], oacc[0:64, :], invb[:])

        def emit_proj(Qp, nt):
            # projection of deferred chunk Qp, one 128-row group; emitted in
            # chunk Qp+1's h1 loop when all of Qp's otp tiles are final
            psy = stp.tile([128, 768], F32, tag="stp", name="psy")
            for co, cw in ((0, 512), (512, 256)):
                for h in range(3):
                    nc.tensor.matmul(
                        psy[:, co:co + cw],
                        lhsT=otp[h][:, nt * 128:(nt + 1) * 128],
                        rhs=pw_sb[h][:, co:co + cw],
                        start=(h == 0),
                        stop=(h == 2),
                    )
            ysb = y_pool.tile([128, 768], F32, tag="ysb", name="ysb")
            nc.vector.tensor_copy(ysb[:], psy[:])
            r0 = Qp * 512 + nt * 128
            nc.sync.dma_start(y[r0:r0 + 128, :], ysb[:])

        def emit_av(h, oacc, bp, pt):
            for j in range(2):
                blk = 2 * bp + j
                nc.tensor.matmul(
                    oacc[:],
                    lhsT=vaug[h][:, blk * 128:(blk + 1) * 128],
                    rhs=pt[:, j * 512:(j + 1) * 512],
                    start=(blk == 0),
                    stop=(blk == NB - 1),
                )

        SCH_A = 184.6649652   # 2^7 / ln(2): bf16-bits Schraudolph slope
        SCH_B = 16248.6       # 127*2^7 minus centering constant

        def emit_qproj(Qn, m):
            sn = slice(Qn * 512, (Qn + 1) * 512)
            psq = stp.tile([128, 512], F32, tag="stp", name="psq")
            for cch in range(6):
                nc.tensor.matmul(
                    psq[:],
                    lhsT=w_sb[cch][m][:],
                    rhs=xq_sb[Qn][cch][:],
                    start=(cch == 0),
                    stop=(cch == 5),
                )
            copyback((q01 if m == 0 else q2)[:, sn], psq[:],
                     wb_sb[:, m:m + 1])

        def emit_sexp(h, bp, qsl, use_dve=False):
            kt, qt = heads[h]
            ps = stp.tile([128, 1024], F32, tag="stp", name="ps")
            for j in range(2):
                blk = 2 * bp + j
                nc.tensor.matmul(
                    ps[:, j * 512:(j + 1) * 512],
                    lhsT=kt[:, blk * 128:(blk + 1) * 128],
                    rhs=qt[:, qsl],
                    start=True,
                    stop=True,
                )
            pt = pt_pool.tile([128, 1024], AT_DT, tag="pt")
            if use_dve:
                # Schraudolph exp on DVE: bf16 bit pattern of exp(x) is
                # approximately round(A*x + B); offloads ACT, which is the
                # pacing engine in steady state
                nc.vector.tensor_scalar(
                    pt[:].bitcast(mybir.dt.int16), ps[:], SCH_A, SCH_B,
                    mybir.AluOpType.mult, mybir.AluOpType.add)
            else:
                nc.scalar.activation(pt[:], ps[:],
                                     mybir.ActivationFunctionType.Exp)
            return pt

        FUSE0 = (NB // 2) == 2 * NS  # chunk-0 attention fused into phase 1
        pendings = []

        # ---- phase 1: full qkv projection, with chunk 0's attention
        # interleaved as its k/v blocks become available ----
        oacc0 = [None] * 3
        av_prev0 = [None] * 3
        xq_sb = [None] * NS
        for s in range(NS):
            ss = slice(s * 512, (s + 1) * 512)
            xs = []
            for cch in range(6):
                t = xp_pool.tile([128, 512], MM_DT, tag=f"xs{cch}_{s}",
                                 name="xs")
                nc.sync.dma_start(t[:], xt[cch * 128:(cch + 1) * 128, ss])
                xs.append(t)
            xq_sb[s] = xs
            for m in (range(5) if s == 0 else (1, 3, 4)):
                ps = stp.tile([128, 512], F32, tag="stp", name="ps")
                for cch in range(6):
                    nc.tensor.matmul(
                        ps[:],
                        lhsT=w_sb[cch][m][:],
                        rhs=xs[cch][:],
                        start=(cch == 0),
                        stop=(cch == 5),
                    )
                if m == 0:
                    copyback(q01[:, ss], ps[:], wb_sb[:, 0:1])
                elif m == 1:
                    copyback(ka[0:64, ss], ps[0:64, :], wb_sb[0:64, 1:2])
                    copyback(kb[64:128, ss], ps[64:128, :],
                             wb_sb[64:128, 1:2])
                elif m == 2:
                    copyback(q2[:, ss], ps[:], wb_sb[:, 2:3])
                elif m == 3:
                    copyback(kc[0:64, ss], ps[0:64, :], wb_sb[0:64, 3:4])
                    vst2 = vst_pool.tile([128, 512], AT_DT, tag="vst",
                                         name="vst2")
                    copyback(vst2[64:128, :], ps[64:128, :],
                             wb_sb[64:128, 3:4])
                else:
                    vst01 = vst_pool.tile([128, 512], AT_DT, tag="vst",
                                          name="vst01")
                    copyback(vst01[:], ps[:], wb_sb[:, 4:5])
            # transpose this chunk's v^T blocks into Vaug
            vsrc = [(vst01, 0), (vst01, 64), (vst2, 64)]
            for h in range(3):
                vs, rb = vsrc[h]
                for j in range(4):
                    blk = 4 * s + j
                    ps = stp.tile([128, 512], F32, tag="stp", name="pst")
                    nc.tensor.matmul(
                        ps[:, 0:64],
                        lhsT=vs[rb:rb + 64, j * 128:(j + 1) * 128],
                        rhs=id_sb[rb:rb + 64, :],
                        start=True,
                        stop=True,
                    )
                    nc.vector.tensor_copy(
                        vaug[h][:, blk * 128:blk * 128 + 64], ps[:, 0:64])
            # fused chunk-0 attention on the freshly produced k/v blocks
            if FUSE0:
                for h in range(3):
                    if s == 0:
                        oacc0[h] = pso.tile([128, 512], F32, tag="pso",
                                            name=f"oacc{h}")
                    for bp in (2 * s, 2 * s + 1):
                        pt = emit_sexp(h, bp, slice(0, 512))
                        if av_prev0[h] is not None:
                            emit_av(h, oacc0[h], bp - 1, av_prev0[h])
                        av_prev0[h] = pt
        if FUSE0:
            for h in range(3):
                emit_av(h, oacc0[h], NB // 2 - 1, av_prev0[h])
                pendings.append((h, oacc0[h], norm_start(oacc0[h])))

        # ---- phase 2: attention + proj per 512-chunk of queries ----
        # pendings (un-finished normalizes) and dproj (un-emitted proj)
        # carry across heads and chunks: head h's normalize finishes ~12
        # blocks into the next head; chunk Q's projection is emitted inside
        # chunk Q+1's h1 loop (otp tiles final, not yet overwritten).
        dproj = 0 if FUSE0 else None
        DEFER = (NB // 2) >= 16
        if NS > 1 and FUSE0:
            emit_qproj(1, 0)
            emit_qproj(1, 2)
        for Q in range(1 if FUSE0 else 0, NS):
            qs = slice(Q * 512, (Q + 1) * 512)
            dqp = Q + 1 if (DEFER and Q + 1 < NS) else None
            if not DEFER and NS > 1 and Q > (0 if not FUSE0 else 1):
                emit_qproj(Q, 0)
                emit_qproj(Q, 2)
            for h in range(3):
                oacc = pso.tile([128, 512], F32, tag="pso", name=f"oacc{h}")
                avq = []
                for bp in range(NB // 2):
                    if pendings and (bp == 12 or
                                     (len(pendings) >= 2 and bp in (4, 8))):
                        norm_finish(*pendings.pop(0))
                    if (DEFER and h == 1 and dproj is not None
                            and bp in (5, 7, 9, 11)):
                        emit_proj(dproj, (bp - 5) // 2)
                        if bp == 11:
                            dproj = None
                    if DEFER and h == 2 and dqp is not None and bp == 12:
                        emit_qproj(dqp, 0)
                    pt = emit_sexp(h, bp, qs)
                    avq.append((bp, pt))
                    if len(avq) > 2:
                        b0, p0 = avq.pop(0)
                        emit_av(h, oacc, b0, p0)
                while avq:
                    b0, p0 = avq.pop(0)
                    emit_av(h, oacc, b0, p0)
                if DEFER and h == 2 and dqp is not None:
                    emit_qproj(dqp, 2)   # after h2's last q2 read
                    dqp = None
                pendings.append((h, oacc, norm_start(oacc)))
            if DEFER:
                dproj = Q
            else:  # small NB: no room to defer, emit eagerly
                while pendings:
                    norm_finish(*pendings.pop(0))
                for nt in range(4):
                    emit_proj(Q, nt)
        # epilogue: remaining normalizes + last chunk's projection
        while pendings:
            norm_finish(*pendings.pop(0))
        if dproj is not None:
            for nt in range(4):
                emit_proj(dproj, nt)

    nc.compile()
    return nc


def host_prep(x, qkv_w, qkv_b, proj_w, seq=N):
    """Build the 8 per-core input maps."""
    f = np.float32
    x = np.asarray(x, f)
    qkv_w = np.asarray(qkv_w, f)
    qkv_b = np.asarray(qkv_b, f)
    proj_w = np.asarray(proj_w, f)

    xts = [np.ascontiguousarray(x[b].T).astype(NP_BF16) for b in range(B)]
    id2 = np.concatenate([np.eye(64, dtype=f)] * 2, axis=0).astype(NP_BF16)

    in_maps = []
    for core in range(8):
        b, g = core // 4, core % 4
        ha, hb_, hc = 3 * g, 3 * g + 1, 3 * g + 2

        def Wrow(base, h):
            return qkv_w[base + h * 64: base + (h + 1) * 64, :]  # [64, 768]

        def brow(base, h):
            return qkv_b[base + h * 64: base + (h + 1) * 64]

        cols = np.concatenate(
            [
                Wrow(0, ha).T * SCALE, Wrow(0, hb_).T * SCALE,   # m0 q01
                Wrow(C, ha).T, Wrow(C, hb_).T,                   # m1 k01
                Wrow(0, hc).T * SCALE, Wrow(0, hc).T * SCALE,    # m2 q2 dup
                Wrow(C, hc).T, Wrow(2 * C, hc).T,                # m3 [k_c|v_c]
                Wrow(2 * C, ha).T, Wrow(2 * C, hb_).T,           # m4 v01
            ],
            axis=1,
        )  # [768, 640]
        bias = np.concatenate(
            [
                brow(0, ha) * SCALE, brow(0, hb_) * SCALE,
                brow(C, ha), brow(C, hb_),
                brow(0, hc) * SCALE, brow(0, hc) * SCALE,
                brow(C, hc), brow(2 * C, hc),
                brow(2 * C, ha), brow(2 * C, hb_),
            ]
        )  # [640]
        wbm = np.zeros((128, 5), f)
        for m in range(5):
            wbm[:, m] = bias[m * 128:(m + 1) * 128]
        pwt = np.zeros((384, 768), f)
        for i, h in enumerate((ha, hb_, hc)):
            pwt[i * 128:i * 128 + 64, :] = proj_w.T[h * 64:(h + 1) * 64, :]

        in_maps.append(
            {
                "xt": xts[b][:, :seq],
                "wqkv": np.ascontiguousarray(cols).astype(NP_BF16),
                "wb": wbm,
                "pwt": pwt.astype(NP_BF16),
                "ident": id2,
            }
        )
    return in_maps


_nc_cache = {}


def _get_nc(seq=N):
    key = seq
    if key not in _nc_cache:
        _nc_cache[key] = build_nc(seq)
    return _nc_cache[key]


def kernel(x, qkv_w, qkv_b, proj_w, proj_b, _trace=False):
    from concourse.bass_utils import run_bass_kernel_spmd

    nc = _get_nc()
    in_maps = host_prep(x, qkv_w, qkv_b, proj_w)
    res = run_bass_kernel_spmd(nc, in_maps, list(range(8)), trace=_trace)
    proj_b = np.asarray(proj_b, np.float32)
    out = np.zeros((B, N, C), np.float32)
    for b in range(B):
        acc = np.zeros((N, C), np.float32)
        for g in range(4):
            acc += res.results[b * 4 + g]["y"]
        out[b] = acc + proj_b[None, :]
    if _trace:
        return out, res
    return out


# revision 34
# speedup vs baseline: 1.1996x; 1.1996x over previous
"""Fused multi-head attention block (B=2, N=4096, C=768, H=12, D=64) for 8
Trainium2 NeuronCores.

Sharding: core c -> (batch b = c // 4, head-group g = c % 4, heads
[3g, 3g+1, 3g+2]).  Megatron-style: qkv weights column-split per head
group, proj weights row-split; each core emits a partial [N, C] output
and the host sums the 4 partials per batch and adds proj_b.

bf16 operand pipeline (PSUM accumulation stays fp32):
  phase 1: k/v^T projection from host-pre-transposed x^T (bf16), x tiles
           kept resident in SBUF for reuse by the phase-2 q projection.
           wqkv column groups (all 128 wide, host-packed):
             m0 [q_a*s | q_b*s]   m1 [k_a | k_b]   m2 [q_c*s | q_c*s]
             m3 [k_c  | v_c ]     m4 [v_a | v_b]
           v^T transposed on-PE (matmul vs identity) into per-key-block
           Vaug slabs [128, 65] whose column 64 is 1.0.
  phase 2: per query-chunk (512 cols), heads processed sequentially so
           each head's softmax-normalize (DVE reciprocal) hides under the
           next head's S/AV matmuls:
             S^T block = kt_blk^T @ qt_chunk (bf16, PSUM fp32)
             P = exp(S) on ACT straight out of PSUM -> bf16
             O'^T[65,512] += Vaug_blk^T @ P^T_blk (PSUM accumulate);
             row 64 = softmax denominator.
           Normalize: DVE reciprocal -> PE outer-product broadcast ->
           DVE multiply -> otp bf16.  proj: y[128,:] = sum_h O^T_h @ pw_h,
           PSUM -> SBUF -> DRAM.
"""

import sys

sys.path.insert(0, "/opt/trn_rl_repo")

from contextlib import ExitStack

import numpy as np
import ml_dtypes

import concourse.bacc as bacc
import concourse.bass as bass
import concourse.mybir as mybir
import concourse.tile as tile

B, N, C, H, D = 2, 4096, 768, 12, 64
SCALE = D ** -0.5
F32 = mybir.dt.float32
F32R = mybir.dt.float32r
BF16 = mybir.dt.bfloat16

MM_DT = BF16
AT_DT = BF16
NP_BF16 = ml_dtypes.bfloat16


def build_nc(seq=N):
    """Build + compile the per-core SPMD program."""
    NS = seq // 512   # 512-wide seq chunks
    NB = seq // 128   # 128-wide kj blocks

    nc = bacc.Bacc("TRN2", target_bir_lowering=False, debug=False, num_devices=8)
    xt = nc.dram_tensor("xt", [768, seq], MM_DT, kind="ExternalInput").ap()
    wqkv = nc.dram_tensor("wqkv", [768, 640], MM_DT, kind="ExternalInput").ap()
    wb = nc.dram_tensor("wb", [128, 5], F32, kind="ExternalInput").ap()
    pwt = nc.dram_tensor("pwt", [384, 768], MM_DT, kind="ExternalInput").ap()
    ident = nc.dram_tensor("ident", [128, 64], MM_DT, kind="ExternalInput").ap()
    y = nc.dram_tensor("y", [seq, 768], F32, kind="ExternalOutput").ap()

    # wqkv column groups (all 128 wide; 640 cols with last 64 unused pad)
    MOFF = [0, 128, 256, 384, 512]

    with tile.TileContext(nc) as tc, ExitStack() as ctx:
        const = ctx.enter_context(tc.tile_pool(name="const", bufs=1))
        big = ctx.enter_context(tc.tile_pool(name="big", bufs=1))
        xp_pool = ctx.enter_context(tc.tile_pool(name="xp", bufs=1))
        pt_pool = ctx.enter_context(tc.tile_pool(name="pt", bufs=12))
        # dedicated pools per normalize temp: a shared ring would chain
        # rinv(h+1)'s allocation behind invb(h)'s last reader (the otp
        # multiply), serializing the reciprocal into the previous head's
        # late normalize chain
        rv_pool = ctx.enter_context(tc.tile_pool(name="rv", bufs=2))
        rr_pool = ctx.enter_context(tc.tile_pool(name="rr", bufs=2))
        ib_pool = ctx.enter_context(tc.tile_pool(name="ib", bufs=2))

        y_pool = ctx.enter_context(tc.tile_pool(name="yp", bufs=2))
        vst_pool = ctx.enter_context(tc.tile_pool(name="vst", bufs=2))
        stp = ctx.enter_context(tc.tile_pool(name="stp", bufs=2, space="PSUM"))
        # 4 oacc slots: chunk Q+1's h0 accumulator must not wait on chunk
        # Q's h0 normalize (deadlock via the in-order PE queue otherwise)
        pso = ctx.enter_context(tc.tile_pool(name="pso", bufs=4, space="PSUM"))

        def copyback(dst, srcap, bias_ap):
            # psum -> sbuf copy with per-partition bias on DVE
            nc.vector.tensor_scalar_add(dst, srcap, bias_ap)

        # ---- PE warmup: get HAM to full clock during the DMA shadow ----
        wscr = const.tile([128, 512], MM_DT, tag="wscr")
        nc.vector.memset(wscr[:], 0.0)
        for i in range(56):
            pw_ = stp.tile([128, 512], F32, tag="stp", name="wup")
            nc.tensor.matmul(pw_[:], lhsT=wscr[:, 0:128], rhs=wscr[:],
                             start=True, stop=True)

        # ---- constants ----
        w_sb = []
        for cch in range(6):
            row = []
            for m in range(5):
                t = const.tile([128, 128], MM_DT, tag=f"w{cch}_{m}",
                               name=f"w{cch}_{m}")
                nc.sync.dma_start(
                    t[:], wqkv[cch * 128:(cch + 1) * 128,
                               m * 128:(m + 1) * 128])
                row.append(t)
            w_sb.append(row)
        wb_sb = const.tile([128, 5], F32, tag="wb")
        nc.sync.dma_start(wb_sb[:], wb[:])
        id_sb = const.tile([128, 64], MM_DT, tag="id")
        nc.gpsimd.dma_start(id_sb[:], ident[:])
        pw_sb = []
        for h in range(3):
            t = const.tile([128, 768], MM_DT, tag=f"pw{h}", name=f"pw{h}")
            nc.sync.dma_start(t[:], pwt[h * 128:(h + 1) * 128, :])
            pw_sb.append(t)
        ones_sb = const.tile([128, 64], F32R, tag="ones")
        nc.vector.memset(ones_sb[:].bitcast(F32), 1.0)

        # ---- persistent qkv^T tensors ----
        q01 = big.tile([128, seq], AT_DT, tag="q01")
        q2 = big.tile([128, seq], AT_DT, tag="q2")
        ka = big.tile([128, seq], AT_DT, tag="ka")
        kb = big.tile([128, seq], AT_DT, tag="kb")
        kc = big.tile([128, seq], AT_DT, tag="kc")
        nc.vector.memset(ka[64:128, :], 0.0)
        nc.vector.memset(kb[0:64, :], 0.0)
        nc.vector.memset(kc[64:128, :], 0.0)
        # Vaug slabs padded to 128 cols (col 64 = ones row-sum column, cols
        # 65:127 harmless pad) so AV weight loads qualify for FWL (128 cols)
        vaug = [big.tile([128, NB * 128], AT_DT, tag=f"va{h}", name=f"va{h}")
                for h in range(3)]
        otp = [big.tile([128, 512], MM_DT, tag=f"otp{h}", name=f"otp{h}")
               for h in range(3)]
        for h in range(3):
            nc.vector.memset(otp[h][64:128, :], 0.0)
        for h in range(3):
            nc.vector.memset(vaug[h][:], 1.0)

        # ---- helpers shared by phase 1 (fused chunk 0) and phase 2 ----
        heads = [(ka, q01), (kb, q01), (kc, q2)]

        def norm_start(oacc):
            # DVE-only part: reciprocal of the softmax sums (slow, ~3.3us)
            # -- runs on DVE without blocking the in-order PE queue
            rinv = rv_pool.tile([128, 512], F32, tag="rinv")
            nc.vector.reciprocal(rinv[64:65, :], oacc[64:65, :])
            rinvr = rr_pool.tile([128, 512], F32R, tag="rinvr", name="rinvr")
            nc.vector.tensor_copy(rinvr[64:65, :], rinv[64:65, :])
            return rinvr

        def norm_finish(h, oacc, rinvr):
            # PE broadcast of 1/sums + DVE multiply -> otp[h] (bf16).
            # Emitted ~4 blocks into the NEXT head so the in-order PE queue
            # reaches it only after the reciprocal chain has finished.
            psb = stp.tile([128, 512], F32, tag="stp", name="psb")
            nc.tensor.matmul(
                psb[0:64, :],
                lhsT=ones_sb[64:65, :],
                rhs=rinvr[64:65, :],
                start=True,
                stop=True,
            )
            invb = ib_pool.tile([64, 512], F32, tag="invb", name="invb")
            nc.vector.tensor_copy(invb[:], psb[0:64, :])
            nc.vector.tensor_mul(otp[h][0:64, :], oacc[0:64, :], invb[:])

        def emit_proj(Qp, nt):
            # projection of deferred chunk Qp, one 128-row group; emitted in
            # chunk Qp+1's h1 loop when all of Qp's otp tiles are final
            psy = stp.tile([128, 768], F32, tag="stp", name="psy")
            for co, cw in ((0, 512), (512, 256)):
                for h in range(3):
                    nc.tensor.matmul(
                        psy[:, co:co + cw],
                        lhsT=otp[h][:, nt * 128:(nt + 1) * 128],
                        rhs=pw_sb[h][:, co:co + cw],
                        start=(h == 0),
                        stop=(h == 2),
                    )
            ysb = y_pool.tile([128, 768], F32, tag="ysb", name="ysb")
            nc.vector.tensor_copy(ysb[:], psy[:])
            r0 = Qp * 512 + nt * 128
            nc.sync.dma_start(y[r0:r0 + 128, :], ysb[:])

        def emit_av(h, oacc, bp, pt):
            for j in range(2):
                blk = 2 * bp + j
                nc.tensor.matmul(
                    oacc[:],
                    lhsT=vaug[h][:, blk * 128:(blk + 1) * 128],
                    rhs=pt[:, j * 512:(j + 1) * 512],
                    start=(blk == 0),
                    stop=(blk == NB - 1),
                )

        SCH_A = 184.6649652   # 2^7 / ln(2): bf16-bits Schraudolph slope
        SCH_B = 16248.6       # 127*2^7 minus centering constant

        def emit_qproj(Qn, m):
            sn = slice(Qn * 512, (Qn + 1) * 512)
            psq = stp.tile([128, 512], F32, tag="stp", name="psq")
            for cch in range(6):
                nc.tensor.matmul(
                    psq[:],
                    lhsT=w_sb[cch][m][:],
                    rhs=xq_sb[Qn][cch][:],
                    start=(cch == 0),
                    stop=(cch == 5),
                )
            copyback((q01 if m == 0 else q2)[:, sn], psq[:],
                     wb_sb[:, m:m + 1])

        def emit_sexp(h, bp, qsl, use_dve=False):
            kt, qt = heads[h]
            ps = stp.tile([128, 1024], F32, tag="stp", name="ps")
            for j in range(2):
                blk = 2 * bp + j
                nc.tensor.matmul(
                    ps[:, j * 512:(j + 1) * 512],
                    lhsT=kt[:, blk * 128:(blk + 1) * 128],
                    rhs=qt[:, qsl],
                    start=True,
                    stop=True,
                )
            pt = pt_pool.tile([128, 1024], AT_DT, tag="pt")
            if use_dve:
                # Schraudolph exp on DVE: bf16 bit pattern of exp(x) is
                # approximately round(A*x + B); offloads ACT, which is the
                # pacing engine in steady state
                nc.vector.tensor_scalar(
                    pt[:].bitcast(mybir.dt.int16), ps[:], SCH_A, SCH_B,
                    mybir.AluOpType.mult, mybir.AluOpType.add)
            else:
                nc.scalar.activation(pt[:], ps[:],
                                     mybir.ActivationFunctionType.Exp)
            return pt

        FUSE0 = (NB // 2) == 2 * NS  # chunk-0 attention fused into phase 1
        pendings = []

        # ---- phase 1: full qkv projection, with chunk 0's attention
        # interleaved as its k/v blocks become available ----
        oacc0 = [None] * 3
        av_prev0 = [None] * 3
        xq_sb = [None] * NS
        for s in range(NS):
            ss = slice(s * 512, (s + 1) * 512)
            xs = []
            for cch in range(6):
                t = xp_pool.tile([128, 512], MM_DT, tag=f"xs{cch}_{s}",
                                 name="xs")
                nc.sync.dma_start(t[:], xt[cch * 128:(cch + 1) * 128, ss])
                xs.append(t)
            xq_sb[s] = xs
            for m in (range(5) if s == 0 else (1, 3, 4)):
                ps = stp.tile([128, 512], F32, tag="stp", name="ps")
                for cch in range(6):
                    nc.tensor.matmul(
                        ps[:],
                        lhsT=w_sb[cch][m][:],
                        rhs=xs[cch][:],
                        start=(cch == 0),
                        stop=(cch == 5),
                    )
                if m == 0:
                    copyback(q01[:, ss], ps[:], wb_sb[:, 0:1])
                elif m == 1:
                    copyback(ka[0:64, ss], ps[0:64, :], wb_sb[0:64, 1:2])
                    copyback(kb[64:128, ss], ps[64:128, :],
                             wb_sb[64:128, 1:2])
                elif m == 2:
                    copyback(q2[:, ss], ps[:], wb_sb[:, 2:3])
                elif m == 3:
                    copyback(kc[0:64, ss], ps[0:64, :], wb_sb[0:64, 3:4])
                    vst2 = vst_pool.tile([128, 512], AT_DT, tag="vst",
                                         name="vst2")
                    copyback(vst2[64:128, :], ps[64:128, :],
                             wb_sb[64:128, 3:4])
                else:
                    vst01 = vst_pool.tile([128, 512], AT_DT, tag="vst",
                                          name="vst01")
                    copyback(vst01[:], ps[:], wb_sb[:, 4:5])
            # transpose this chunk's v^T blocks into Vaug
            vsrc = [(vst01, 0), (vst01, 64), (vst2, 64)]
            for h in range(3):
                vs, rb = vsrc[h]
                for j in range(4):
                    blk = 4 * s + j
                    ps = stp.tile([128, 512], F32, tag="stp", name="pst")
                    nc.tensor.matmul(
                        ps[:, 0:64],
                        lhsT=vs[rb:rb + 64, j * 128:(j + 1) * 128],
                        rhs=id_sb[rb:rb + 64, :],
                        start=True,
                        stop=True,
                    )
                    nc.vector.tensor_copy(
                        vaug[h][:, blk * 128:blk * 128 + 64], ps[:, 0:64])
            # fused chunk-0 attention on the freshly produced k/v blocks
            if FUSE0:
                for h in range(3):
                    if s == 0:
                        oacc0[h] = pso.tile([128, 512], F32, tag="pso",
                                            name=f"oacc{h}")
                    for bp in (2 * s, 2 * s + 1):
                        pt = emit_sexp(h, bp, slice(0, 512))
                        if av_prev0[h] is not None:
                            emit_av(h, oacc0[h], bp - 1, av_prev0[h])
                        av_prev0[h] = pt
        if FUSE0:
            for h in range(3):
                emit_av(h, oacc0[h], NB // 2 - 1, av_prev0[h])
                pendings.append((h, oacc0[h], norm_start(oacc0[h])))

        # ---- phase 2: attention + proj per 512-chunk of queries ----
        # pendings (un-finished normalizes) and dproj (un-emitted proj)
        # carry across heads and chunks: head h's normalize finishes ~12
        # blocks into the next head; chunk Q's projection is emitted inside
        # chunk Q+1's h1 loop (otp tiles final, not yet overwritten).
        dproj = 0 if FUSE0 else None
        DEFER = (NB // 2) >= 16
        if NS > 1 and FUSE0:
            emit_qproj(1, 0)
            emit_qproj(1, 2)
        for Q in range(1 if FUSE0 else 0, NS):
            qs = slice(Q * 512, (Q + 1) * 512)
            dqp = Q + 1 if (DEFER and Q + 1 < NS) else None
            if not DEFER and NS > 1 and Q > (0 if not FUSE0 else 1):
                emit_qproj(Q, 0)
                emit_qproj(Q, 2)
            for h in range(3):
                oacc = pso.tile([128, 512], F32, tag="pso", name=f"oacc{h}")
                avq = []
                for bp in range(NB // 2):
                    if pendings and (bp == 12 or
                                     (len(pendings) >= 2 and bp in (4, 8))):
                        norm_finish(*pendings.pop(0))
                    if (DEFER and h == 1 and dproj is not None
                            and bp in (5, 7, 9, 11)):
                        emit_proj(dproj, (bp - 5) // 2)
                        if bp == 11:
                            dproj = None
                    if DEFER and h == 2 and dqp is not None and bp == 12:
                        emit_qproj(dqp, 0)
                    pt = emit_sexp(h, bp, qs, use_dve=bp in (6, 13))
                    avq.append((bp, pt))
                    if len(avq) > 2:
                        b0, p0 = avq.pop(0)
                        emit_av(h, oacc, b0, p0)
                while avq:
                    b0, p0 = avq.pop(0)
                    emit_av(h, oacc, b0, p0)
                if DEFER and h == 2 and dqp is not None:
                    emit_qproj(dqp, 2)   # after h2's last q2 read
                    dqp = None
                pendings.append((h, oacc, norm_start(oacc)))
            if DEFER:
                dproj = Q
            else:  # small NB: no room to defer, emit eagerly
                while pendings:
                    norm_finish(*pendings.pop(0))
                for nt in range(4):
                    emit_proj(Q, nt)
        # epilogue: remaining normalizes + last chunk's projection
        while pendings:
            norm_finish(*pendings.pop(0))
        if dproj is not None:
            for nt in range(4):
                emit_proj(dproj, nt)

    nc.compile()
    return nc


def host_prep(x, qkv_w, qkv_b, proj_w, seq=N):
    """Build the 8 per-core input maps."""
    f = np.float32
    x = np.asarray(x, f)
    qkv_w = np.asarray(qkv_w, f)
    qkv_b = np.asarray(qkv_b, f)
    proj_w = np.asarray(proj_w, f)

    xts = [np.ascontiguousarray(x[b].T).astype(NP_BF16) for b in range(B)]
    id2 = np.concatenate([np.eye(64, dtype=f)] * 2, axis=0).astype(NP_BF16)

    in_maps = []
    for core in range(8):
        b, g = core // 4, core % 4
        ha, hb_, hc = 3 * g, 3 * g + 1, 3 * g + 2

        def Wrow(base, h):
            return qkv_w[base + h * 64: base + (h + 1) * 64, :]  # [64, 768]

        def brow(base, h):
            return qkv_b[base + h * 64: base + (h + 1) * 64]

        cols = np.concatenate(
            [
                Wrow(0, ha).T * SCALE, Wrow(0, hb_).T * SCALE,   # m0 q01
                Wrow(C, ha).T, Wrow(C, hb_).T,                   # m1 k01
                Wrow(0, hc).T * SCALE, Wrow(0, hc).T * SCALE,    # m2 q2 dup
                Wrow(C, hc).T, Wrow(2 * C, hc).T,                # m3 [k_c|v_c]
                Wrow(2 * C, ha).T, Wrow(2 * C, hb_).T,           # m4 v01
            ],
            axis=1,
        )  # [768, 640]
        bias = np.concatenate(
            [
                brow(0, ha) * SCALE, brow(0, hb_) * SCALE,
                brow(C, ha), brow(C, hb_),
                brow(0, hc) * SCALE, brow(0, hc) * SCALE,
                brow(C, hc), brow(2 * C, hc),
                brow(2 * C, ha), brow(2 * C, hb_),
            ]
        )  # [640]
        wbm = np.zeros((128, 5), f)
        for m in range(5):
            wbm[:, m] = bias[m * 128:(m + 1) * 128]
        pwt = np.zeros((384, 768), f)
        for i, h in enumerate((ha, hb_, hc)):
            pwt[i * 128:i * 128 + 64, :] = proj_w.T[h * 64:(h + 1) * 64, :]

        in_maps.append(
            {
                "xt": xts[b][:, :seq],
                "wqkv": np.ascontiguousarray(cols).astype(NP_BF16),
                "wb": wbm,
                "pwt": pwt.astype(NP_BF16),
                "ident": id2,
            }
        )
    return in_maps


_nc_cache = {}


def _get_nc(seq=N):
    key = seq
    if key not in _nc_cache:
        _nc_cache[key] = build_nc(seq)
    return _nc_cache[key]


def kernel(x, qkv_w, qkv_b, proj_w, proj_b, _trace=False):
    from concourse.bass_utils import run_bass_kernel_spmd

    nc = _get_nc()
    in_maps = host_prep(x, qkv_w, qkv_b, proj_w)
    res = run_bass_kernel_spmd(nc, in_maps, list(range(8)), trace=_trace)
    proj_b = np.asarray(proj_b, np.float32)
    out = np.zeros((B, N, C), np.float32)
    for b in range(B):
        acc = np.zeros((N, C), np.float32)
        for g in range(4):
            acc += res.results[b * 4 + g]["y"]
        out[b] = acc + proj_b[None, :]
    if _trace:
        return out, res
    return out
